# revision 1
# baseline (speedup 1.0000x reference)
"""GAT forward on 8 Trainium2 NeuronCores — one attention head per core.

Math (per head, all [4096] nodes):
    h   = x @ W                      [N, 128]
    ci  = h @ w_i  (per-node)        [N]
    cj  = h @ w_j  (per-node)        [N]
    e^T[j, i] = exp(leaky_relu(ci[i] + cj[j] + M[j, i]))   (M = 0 / -1e9 additive mask,
                M[j, i] = -1e9 where graph[j, i] == 0; masked entries exp to exactly 0)
    yT[f, i] = sum_j h[j, f] * eT[j, i]        (PE matmul, e as moving operand)
    rs[i]    = sum_j eT[j, i]                  (PE matmul vs ones column)
    y[i, f]  = yT[f, i] / rs[i] + (x @ W_r_head)[i, f]     (+ bias on host)

Layout/scheduling notes:
  - Scores are computed TRANSPOSED (j on partitions) so the adjacency mask loads
    in natural row order and e feeds the PE as the moving operand.
  - i is split in two 2048-wide halves so PSUM holds yT-half (4 banks) + rowsum
    (4 banks) simultaneously. Both halves' normalize/transpose finales are
    deferred past the second j-loop so the PE/ACT pipeline never stalls on the
    PSUM handoff mid-kernel.
  - Elementwise softmax numerator: DVE scalar_tensor_tensor (ciB + cj[j]) + M,
    then leaky-relu (ACT Prelu alpha=0.2 for most tiles, DVE mul+max for a
    fraction to balance engines), then ACT Exp -> float32r.
  - Projections go through hT[f, j] / residT[f, i] (N=512 fp32 matmuls); h and
    resid are recovered with PE transposes. float32r is used wherever the
    producer is a compute op (hT, h, e, ones). Phase-1 PSUM works in
    [128, 2048] half-tiles with 2 pool slots so evacuations overlap matmuls.
"""
import sys

sys.path.insert(0, "/opt/trn_rl_repo")
from contextlib import ExitStack

import numpy as np
import ml_dtypes

import concourse.bass as bass
import concourse.tile as tile
from concourse import bacc, mybir
from concourse.bass_utils import run_bass_kernel_spmd

dt = mybir.dt
F32, F32R, BF16 = dt.float32, dt.float32r, dt.bfloat16
AF = mybir.ActivationFunctionType
OP = mybir.AluOpType

N = 4096
IN_F = 512
HF = 128
HEADS = 8
SLOPE = 0.2
MASK_NEG = -1.0e9
HALF = 2048
NJT = N // 128  # 32 j-tiles
NMC = IN_F // 128  # 4 contraction chunks over in-features

DVE_LRELU_MOD = 4  # j-tiles with jt % MOD == 3 do leaky-relu on DVE instead of ACT

_prog = None


def build_program():
    nc = bacc.Bacc("TRN2", target_bir_lowering=False, debug=False)
    xT_d = nc.dram_tensor("xT", [IN_F, N], F32, kind="ExternalInput").ap()
    mask_d = nc.dram_tensor("mask", [N, N], BF16, kind="ExternalInput").ap()
    W_d = nc.dram_tensor("W", [IN_F, HF], F32, kind="ExternalInput").ap()
    Wr_d = nc.dram_tensor("Wr", [IN_F, HF], F32, kind="ExternalInput").ap()
    wi_d = nc.dram_tensor("wi", [HF, 1], F32, kind="ExternalInput").ap()
    wj_d = nc.dram_tensor("wj", [HF, 1], F32, kind="ExternalInput").ap()
    eye_d = nc.dram_tensor("eye", [128, 128], F32, kind="ExternalInput").ap()
    y_d = nc.dram_tensor("y", [N, HF], F32, kind="ExternalOutput").ap()

    with tile.TileContext(nc) as tc, ExitStack() as ctx:
        persist = ctx.enter_context(tc.tile_pool(name="persist", bufs=1))
        h_sb = persist.tile([128, N], F32R, tag="h")  # h[j,f], slice jt -> j-tile
        resid_sb = persist.tile([128, N], F32, tag="resid")  # resid[i,f] per i-tile
        ciB = persist.tile([128, N], F32, tag="ciB")  # ci broadcast along partitions
        cjT = persist.tile([128, 2 * NJT], F32, tag="cjT")  # cj[j] cols (even idx)
        eye_sb = persist.tile([128, 128], F32, tag="eye")
        ones_r = persist.tile([128, 1], F32R, tag="ones")

        nc.sync.dma_start(eye_sb[:], eye_d)
        ones_f = persist.tile([128, 1], F32, tag="ones_f")
        nc.vector.memset(ones_f[:], 1.0)
        nc.vector.tensor_copy(ones_r[:], ones_f[:])
        eye_r = persist.tile([128, 128], F32R, tag="eye_r")
        nc.vector.tensor_copy(eye_r[:], eye_sb[:])

        # Phase-2 pools opened FIRST: their SBUF is disjoint from phase-1
        # buffers, so attention tiles never wait on projection-buffer releases.
        ph2 = ctx.enter_context(tc.tile_pool(name="ph2", bufs=3))
        inpool = ctx.enter_context(tc.tile_pool(name="inpool", bufs=4))
        epool = ctx.enter_context(tc.tile_pool(name="epool", bufs=3))
        tpool = ctx.enter_context(tc.tile_pool(name="tpool", bufs=1))
        fin = ctx.enter_context(tc.tile_pool(name="fin", bufs=2))
        outp = ctx.enter_context(tc.tile_pool(name="outp", bufs=2))

        # ---------- Phase 1: hT[f,j] + resid[i,f] interleaved over streamed xT ----------
        with ExitStack() as p1:
            ph1 = p1.enter_context(tc.tile_pool(name="ph1", bufs=1))
            xpool = p1.enter_context(tc.tile_pool(name="xpool", bufs=2))
            psb = p1.enter_context(tc.tile_pool(name="psb", bufs=1, space="PSUM"))

            W_sb = ph1.tile([128, NMC * HF], F32, tag="W")
            Wr_sb = ph1.tile([128, NMC * HF], F32, tag="Wr")
            for mc in range(NMC):
                nc.sync.dma_start(
                    W_sb[:, mc * HF : (mc + 1) * HF], W_d[mc * 128 : (mc + 1) * 128, :]
                )
                nc.sync.dma_start(
                    Wr_sb[:, mc * HF : (mc + 1) * HF],
                    Wr_d[mc * 128 : (mc + 1) * 128, :],
                )
            wi_sb = ph1.tile([128, 1], F32, tag="wi")
            nc.sync.dma_start(wi_sb[:], wi_d)
            wj_sb = ph1.tile([128, 1], F32, tag="wj")
            nc.sync.dma_start(wj_sb[:], wj_d)
            wi_r = ph1.tile([128, 1], F32R, tag="wi_r")
            nc.vector.tensor_copy(wi_r[:], wi_sb[:])
            # wj padded to 2 columns: f32r matmuls need an even moving free dim
            wj2_f = ph1.tile([128, 2], F32, tag="wj2_f")
            nc.vector.memset(wj2_f[:], 0.0)
            nc.vector.tensor_copy(wj2_f[:, 0:1], wj_sb[:])
            wj_r = ph1.tile([128, 2], F32R, tag="wj_r")
            nc.vector.tensor_copy(wj_r[:], wj2_f[:])

            hT_sb = ph1.tile([128, N], F32R, tag="hT")  # hT[f, j]

            for hf in range(2):
                o = hf * HALF
                ps_hT = psb.tile([128, HALF], F32, tag="psA")
                ps_res = psb.tile([128, HALF], F32, tag="psB")
                for mc in range(NMC):
                    for ck in range(2):
                        oc = ck * 1024
                        xt = xpool.tile([128, 1024], F32, tag="xt")
                        nc.sync.dma_start(
                            xt[:],
                            xT_d[mc * 128 : (mc + 1) * 128, o + oc : o + oc + 1024],
                        )
                        for nck in range(2):
                            nc.tensor.matmul(
                                ps_hT[:, oc + nck * 512 : oc + (nck + 1) * 512],
                                W_sb[:, mc * HF : (mc + 1) * HF],
                                xt[:, nck * 512 : (nck + 1) * 512],
                                start=(mc == 0),
                                stop=(mc == NMC - 1),
                            )
                        for it in range(8):
                            git = ck * 8 + it
                            nc.tensor.matmul(
                                ps_res[:, oc + it * 128 : oc + (it + 1) * 128],
                                xt[:, it * 128 : (it + 1) * 128],
                                Wr_sb[:, mc * HF : (mc + 1) * HF],
                                start=(mc == 0 and git % 4 == 0),
                                stop=(mc == NMC - 1),
                            )
                for nck in range(HALF // 512):
                    nc.vector.tensor_copy(
                        hT_sb[:, o + nck * 512 : o + (nck + 1) * 512],
                        ps_hT[:, nck * 512 : (nck + 1) * 512],
                    )
                nc.scalar.copy(resid_sb[:, o : o + HALF], ps_res[:])

                # ci for this half -> broadcast that half of ciB immediately
                ps_ci = psb.tile([1, HALF], F32, tag="psA")
                for nck in range(HALF // 512):
                    nc.tensor.matmul(
                        ps_ci[0:1, nck * 512 : (nck + 1) * 512],
                        wi_r[:],
                        hT_sb[:, o + nck * 512 : o + (nck + 1) * 512],
                        start=True,
                        stop=True,
                    )
                ci_rowh = ph1.tile([1, HALF], F32, tag="ci_row")
                nc.vector.tensor_copy(ci_rowh[:], ps_ci[:])
                nc.gpsimd.partition_broadcast(
                    ciB[:, o : o + HALF], ci_rowh[0:1, :]
                )

                # cj columns for this half of j-tiles
                ps_cj = psb.tile([128, NJT], F32, tag="psB")
                for k in range(NJT // 2):
                    jt = hf * (NJT // 2) + k
                    nc.tensor.matmul(
                        ps_cj[:, 2 * k : 2 * k + 2],
                        hT_sb[:, jt * 128 : (jt + 1) * 128],
                        wj_r[:],
                        start=(k == 0),
                        stop=(k == NJT // 2 - 1),
                    )
                nc.vector.tensor_copy(
                    cjT[:, hf * NJT : (hf + 1) * NJT], ps_cj[:]
                )

                # h[j, f] for this half of j-tiles = transpose(hT) blockwise
                ps_h = psb.tile([128, HALF], F32R, tag="psA")
                for k in range(HALF // 128):
                    jt = hf * (HALF // 128) + k
                    nc.tensor.transpose(
                        ps_h[:, k * 128 : (k + 1) * 128],
                        hT_sb[:, jt * 128 : (jt + 1) * 128],
                        eye_r[:],
                    )
                nc.scalar.copy(h_sb[:, o : o + HALF], ps_h[:])

        # ---------- Phase 2: attention ----------

        for half in range(2):
            i0 = half * HALF
            with ExitStack() as pmm_ctx:
                pmm = pmm_ctx.enter_context(
                    tc.tile_pool(name=f"pmm{half}", bufs=1, space="PSUM")
                )
                yT_ps = pmm.tile([128, HALF], F32, tag="yT")
                rs_ps = pmm.tile([1, HALF], F32, tag="rs")

                for jt in range(NJT):
                    m_t = ph2.tile([128, HALF], BF16, tag="m")
                    nc.sync.dma_start(
                        m_t[:], mask_d[jt * 128 : (jt + 1) * 128, i0 : i0 + HALF]
                    )
                    IN = inpool.tile([128, HALF], F32, tag="IN")
                    nc.vector.scalar_tensor_tensor(
                        IN[:],
                        ciB[:, i0 : i0 + HALF],
                        cjT[:, (jt // (NJT // 2)) * NJT + 2 * (jt % (NJT // 2)) : (jt // (NJT // 2)) * NJT + 2 * (jt % (NJT // 2)) + 1],
                        m_t[:],
                        op0=OP.add,
                        op1=OP.add,
                    )
                    # leaky-relu split across engines per tile: ACT takes the
                    # first 1536 columns (Prelu), DVE the last 512 (mul+max) --
                    # uniform per-tile latency keeps the PE warm
                    DSP = HALF - 384
                    nc.scalar.activation(
                        IN[:, 0:DSP], IN[:, 0:DSP], AF.Prelu, alpha=SLOPE
                    )
                    t_t = tpool.tile([128, 384], F32, tag="t")
                    nc.vector.tensor_scalar_mul(t_t[:], IN[:, DSP:HALF], SLOPE)
                    nc.vector.tensor_max(IN[:, DSP:HALF], IN[:, DSP:HALF], t_t[:])
                    e_r = epool.tile([128, HALF], F32R, tag="e")
                    nc.scalar.activation(e_r[:], IN[:], AF.Exp)

                    hr = h_sb[:, jt * 128 : (jt + 1) * 128]
                    for c in range(HALF // 512):
                        nc.tensor.matmul(
                            yT_ps[:, c * 512 : (c + 1) * 512],
                            hr,
                            e_r[:, c * 512 : (c + 1) * 512],
                            start=(jt == 0),
                            stop=(jt == NJT - 1),
                        )
                    for c in range(HALF // 512):
                        nc.tensor.matmul(
                            rs_ps[0:1, c * 512 : (c + 1) * 512],
                            ones_r[:],
                            e_r[:, c * 512 : (c + 1) * 512],
                            start=(jt == 0),
                            stop=(jt == NJT - 1),
                        )

                yT_sb = fin.tile([128, HALF], F32, tag="yT_sb")
                nc.vector.tensor_copy(yT_sb[:], yT_ps[:])
                rs_sb = fin.tile([1, HALF], F32, tag="rs_sb")
                nc.scalar.copy(rs_sb[:], rs_ps[:])

            # per-half finale: brief PSUM use between the two halves
            with ExitStack() as pf_ctx:
                pfin = pf_ctx.enter_context(
                    tc.tile_pool(name=f"pfin{half}", bufs=1, space="PSUM")
                )
                rsT_ps = pfin.tile([128, HALF // 128], F32, tag="rsT")
                for c in range(HALF // 128):
                    nc.tensor.transpose(
                        rsT_ps[:, c : c + 1],
                        rs_sb[0:1, c * 128 : (c + 1) * 128],
                        eye_sb[0:1, 0:1],
                    )
                rsT_sb = fin.tile([128, HALF // 128], F32, tag="rsT_sb")
                nc.vector.tensor_copy(rsT_sb[:], rsT_ps[:])
                recipT = fin.tile([128, HALF // 128], F32, tag="recipT")
                nc.vector.reciprocal(recipT[:], rsT_sb[:])

                tr_ps = pfin.tile([128, HALF], F32, tag="tr")
                for gi in range(HALF // 128):
                    nc.tensor.transpose(
                        tr_ps[:, gi * 128 : (gi + 1) * 128],
                        yT_sb[:, gi * 128 : (gi + 1) * 128],
                        eye_sb[:],
                    )
                # evacuate transposed y to SBUF so the PSUM banks free for the
                # next half's accumulation; combines run during that half
                ytr_sb = fin.tile([128, HALF], F32, tag="ytr_sb")
                nc.vector.tensor_copy(ytr_sb[:], tr_ps[:])
            for gi in range(HALF // 128):
                g = half * (HALF // 128) + gi
                ob = outp.tile([128, HF], F32, tag="ob")
                nc.vector.scalar_tensor_tensor(
                    ob[:],
                    ytr_sb[:, gi * 128 : (gi + 1) * 128],
                    recipT[:, gi : gi + 1],
                    resid_sb[:, g * 128 : (g + 1) * 128],
                    op0=OP.mult,
                    op1=OP.add,
                )
                nc.sync.dma_start(y_d[g * 128 : (g + 1) * 128, :], ob[:])

    nc.compile()
    return nc


def _get_program():
    global _prog
    if _prog is None:
        _prog = build_program()
    return _prog


def _prepare_in_maps(x, graph, W, w_i, w_j, W_r):
    xT = np.ascontiguousarray(x.T).astype(np.float32, copy=False)
    mask = np.where(graph > 0, np.float32(0.0), np.float32(MASK_NEG)).astype(
        ml_dtypes.bfloat16
    )
    eye = np.eye(128, dtype=np.float32)
    in_maps = []
    for c in range(HEADS):
        in_maps.append(
            {
                "xT": xT,
                "mask": mask,
                "W": np.ascontiguousarray(W[c]).astype(np.float32, copy=False),
                "Wr": np.ascontiguousarray(W_r[:, c * HF : (c + 1) * HF]).astype(
                    np.float32, copy=False
                ),
                "wi": np.ascontiguousarray(w_i[c]).astype(np.float32, copy=False),
                "wj": np.ascontiguousarray(w_j[c]).astype(np.float32, copy=False),
                "eye": eye,
            }
        )
    return in_maps


def run(inputs, trace=False, **kwargs):
    """Run the SPMD kernel; returns (y_full, BassKernelResults)."""
    x = np.asarray(inputs["x"], dtype=np.float32)
    graph = np.asarray(inputs["graph"])
    W = np.asarray(inputs["W"], dtype=np.float32)
    w_i = np.asarray(inputs["w_i"], dtype=np.float32)
    w_j = np.asarray(inputs["w_j"], dtype=np.float32)
    W_r = np.asarray(inputs["W_r"], dtype=np.float32)
    bias = np.asarray(inputs["bias"], dtype=np.float32)

    nc = _get_program()
    in_maps = _prepare_in_maps(x, graph, W, w_i, w_j, W_r)
    br = run_bass_kernel_spmd(
        nc, in_maps, core_ids=list(range(HEADS)), trace=trace, **kwargs
    )
    y = np.concatenate([br.results[c]["y"] for c in range(HEADS)], axis=1)
    y = y + bias[None, :]
    return y.astype(np.float32), br


def kernel(**inputs):
    y, _ = run(inputs)
    return y



# revision 12
# speedup vs baseline: 1.3159x; 1.3159x over previous
"""GAT forward on 8 Trainium2 NeuronCores — one attention head per core.

Math (per head, all [4096] nodes):
    h    = x @ W                    [N, 128]
    ci   = h @ w_i  (per-node)      [N]
    cj   = h @ w_j  (per-node)      [N]
    z    = leaky_relu(ci[i] + cj[j] + m[j,i])   (m = 0 / -60 additive mask)
    e    = exp(z)  (Schraudolph bit-trick on DVE, bf16)
    yT[f,i] = sum_j h[j,f] * e[j,i]            (PE matmul, e moving)
    rs[i]   = sum_j e[j,i]                     (PE matmul vs ones4 stationary)
    out[f,i] = yT[f,i]/rs[i] + residT[f,i];  host transposes to [i,f].

Engine assignment per attention tile [128j x 2048i] (64 tiles):
  - DVE:  w = mask + ciB          (tensor_tensor bf16, 2x mode)
          e = int16(z*A + B)      (tensor_scalar bf16->int16, Schraudolph exp;
                                   bitcast to bf16 feeds the PE directly)
  - ACT:  z = Prelu(w + cj_bias)  (one op does the cj add AND the leaky relu)
  - PE:   4x yT matmuls + 4x rowsum matmuls (all bf16, 512-col chunks)
  - Pool: mask DMA issue + ci/recip partition broadcasts
All matmuls everywhere are bf16 (1 cyc/row); phase 1 computes hT and residT
with the same streamed xT tiles. rowsum lands in one PSUM bank via a [4,512]
layout (block-diagonal ones stationary). Host does transpose/concat/bias.
"""
import sys

sys.path.insert(0, "/opt/trn_rl_repo")
from contextlib import ExitStack

import numpy as np
import ml_dtypes

import concourse.bass as bass
import concourse.tile as tile
from concourse import bacc, mybir
from concourse.bass_utils import run_bass_kernel_spmd

dt = mybir.dt
F32, BF16, I16 = dt.float32, dt.bfloat16, dt.int16
AF = mybir.ActivationFunctionType
OP = mybir.AluOpType

N = 4096
IN_F = 512
HF = 128
HEADS = 8
SLOPE = 0.2
MASK_NEG = -60.0
HALF = 2048
NJT = N // 128  # 32 j-tiles
NMC = IN_F // 128  # 4 contraction chunks
A_EXP = 184.6650292  # 128 * log2(e)
B_EXP = 16248.58  # 127*128 - schraudolph correction (round-to-nearest)

_prog = None


def build_program():
    nc = bacc.Bacc("TRN2", target_bir_lowering=False, debug=False)
    xT_d = nc.dram_tensor("xT", [IN_F, N], BF16, kind="ExternalInput").ap()
    mask_d = nc.dram_tensor("mask", [N, N], BF16, kind="ExternalInput").ap()
    W_d = nc.dram_tensor("W", [IN_F, HF], BF16, kind="ExternalInput").ap()
    Wr_d = nc.dram_tensor("Wr", [IN_F, HF], BF16, kind="ExternalInput").ap()
    wi_d = nc.dram_tensor("wi", [HF, 1], BF16, kind="ExternalInput").ap()
    wj2_d = nc.dram_tensor("wj2", [HF, 2], BF16, kind="ExternalInput").ap()
    ones_d = nc.dram_tensor("ones", [128, 1], BF16, kind="ExternalInput").ap()
    eye_d = nc.dram_tensor("eye", [128, 128], BF16, kind="ExternalInput").ap()
    y_d = nc.dram_tensor("y", [HF, N], BF16, kind="ExternalOutput").ap()

    with tile.TileContext(nc) as tc, ExitStack() as ctx:
        persist = ctx.enter_context(tc.tile_pool(name="persist", bufs=1))
        h_sb = persist.tile([128, N], BF16, tag="h")  # h[j,f] per j-tile
        rT_sb = persist.tile([128, N], BF16, tag="rT")  # residT[f,i]
        ciB = persist.tile([128, N], BF16, tag="ciB")  # ci bcast along partitions
        cjT = persist.tile([128, 2 * NJT], F32, tag="cjT")  # cj at even cols
        ones_sb = persist.tile([128, 1], BF16, tag="ones")
        eye_sb = persist.tile([128, 128], BF16, tag="eye")
        nc.sync.dma_start(ones_sb[:], ones_d)
        nc.sync.dma_start(eye_sb[:], eye_d)

        # Phase-2 pools opened first so their SBUF is disjoint from phase-1
        # scoped buffers.
        mpool = ctx.enter_context(tc.tile_pool(name="mpool", bufs=6))
        wpool = ctx.enter_context(tc.tile_pool(name="wpool", bufs=3))
        zpool = ctx.enter_context(tc.tile_pool(name="zpool", bufs=3))
        epool = ctx.enter_context(tc.tile_pool(name="epool", bufs=3))
        fin = ctx.enter_context(tc.tile_pool(name="fin", bufs=2))

        # ---------- Phase 1: hT/residT over streamed xT; ci/cj/h ----------
        with ExitStack() as p1:
            ph1 = p1.enter_context(tc.tile_pool(name="ph1", bufs=1))
            xpool = p1.enter_context(tc.tile_pool(name="xpool", bufs=3))
            hTp = p1.enter_context(tc.tile_pool(name="hTp", bufs=2))

            W_sb = ph1.tile([128, NMC * HF], BF16, tag="W")
            Wr_sb = ph1.tile([128, NMC * HF], BF16, tag="Wr")
            for mc in range(NMC):
                nc.sync.dma_start(
                    W_sb[:, mc * HF : (mc + 1) * HF], W_d[mc * 128 : (mc + 1) * 128, :]
                )
                nc.sync.dma_start(
                    Wr_sb[:, mc * HF : (mc + 1) * HF],
                    Wr_d[mc * 128 : (mc + 1) * 128, :],
                )
            wi_sb = ph1.tile([128, 1], BF16, tag="wi")
            nc.sync.dma_start(wi_sb[:], wi_d)
            wj2_sb = ph1.tile([128, 2], BF16, tag="wj2")
            nc.sync.dma_start(wj2_sb[:], wj2_d)

            for hf in range(2):
                o = hf * HALF
                hT_sb = hTp.tile([128, HALF], BF16, tag="hT")
                with ExitStack() as ps1:
                    psA = ps1.enter_context(
                        tc.tile_pool(name=f"psA{hf}", bufs=1, space="PSUM")
                    )
                    ps_hT = psA.tile([128, HALF], F32, tag="ps_hT")
                    ps_rT = psA.tile([128, HALF], F32, tag="ps_rT")
                    for mc in range(NMC):
                        xt = xpool.tile([128, HALF], BF16, tag="xt")
                        nc.sync.dma_start(
                            xt[:], xT_d[mc * 128 : (mc + 1) * 128, o : o + HALF]
                        )
                        for ck in range(HALF // 512):
                            nc.tensor.matmul(
                                ps_hT[:, ck * 512 : (ck + 1) * 512],
                                W_sb[:, mc * HF : (mc + 1) * HF],
                                xt[:, ck * 512 : (ck + 1) * 512],
                                start=(mc == 0),
                                stop=(mc == NMC - 1),
                            )
                        for ck in range(HALF // 512):
                            nc.tensor.matmul(
                                ps_rT[:, ck * 512 : (ck + 1) * 512],
                                Wr_sb[:, mc * HF : (mc + 1) * HF],
                                xt[:, ck * 512 : (ck + 1) * 512],
                                start=(mc == 0),
                                stop=(mc == NMC - 1),
                            )
                    nc.vector.tensor_copy(hT_sb[:], ps_hT[:])
                    nc.scalar.copy(rT_sb[:, o : o + HALF], ps_rT[:])

                with ExitStack() as ps2:
                    psB = ps2.enter_context(
                        tc.tile_pool(name=f"psB{hf}", bufs=1, space="PSUM")
                    )
                    ps_ci = psB.tile([1, HALF], F32, tag="ps_ci")
                    for c in range(4):
                        nc.tensor.matmul(
                            ps_ci[0:1, c * 512 : (c + 1) * 512],
                            wi_sb[:],
                            hT_sb[:, c * 512 : (c + 1) * 512],
                            start=True,
                            stop=True,
                        )
                    ci_row = ph1.tile([1, HALF], BF16, tag=f"ci_row{hf}")
                    nc.vector.tensor_copy(ci_row[:], ps_ci[:])
                    for c in range(4):
                        nc.gpsimd.partition_broadcast(
                            ciB[:, o + c * 512 : o + (c + 1) * 512],
                            ci_row[0:1, c * 512 : (c + 1) * 512],
                        )

                    ps_cj = psB.tile([128, NJT], F32, tag="ps_cj")
                    for k in range(NJT // 2):
                        jt = hf * (NJT // 2) + k
                        nc.tensor.matmul(
                            ps_cj[:, 2 * k : 2 * k + 2],
                            hT_sb[:, k * 128 : (k + 1) * 128],
                            wj2_sb[:],
                            start=(k == 0),
                            stop=(k == NJT // 2 - 1),
                        )
                    nc.vector.tensor_copy(cjT[:, hf * NJT : (hf + 1) * NJT], ps_cj[:])

                    ps_h = psB.tile([128, HALF], BF16, tag="ps_h")
                    for k in range(HALF // 128):
                        nc.tensor.transpose(
                            ps_h[:, k * 128 : (k + 1) * 128],
                            hT_sb[:, k * 128 : (k + 1) * 128],
                            eye_sb[:],
                        )
                    nc.scalar.copy(h_sb[:, o : o + HALF], ps_h[:])

        # ---------- Phase 2: attention ----------
        for half in range(2):
            i0 = half * HALF
            with ExitStack() as pmm_ctx:
                pmm = pmm_ctx.enter_context(
                    tc.tile_pool(name=f"pmm{half}", bufs=1, space="PSUM")
                )
                yT_ps = pmm.tile([128, HALF], F32, tag="yT")
                rs_ps = pmm.tile([1, HALF], F32, tag="rs")

                for jt in range(NJT):
                    m_t = mpool.tile([128, HALF], BF16, tag="m")
                    nc.gpsimd.dma_start(
                        m_t[:], mask_d[jt * 128 : (jt + 1) * 128, i0 : i0 + HALF]
                    )
                    w_t = wpool.tile([128, HALF], BF16, tag="w")
                    nc.vector.tensor_tensor(
                        w_t[:], m_t[:], ciB[:, i0 : i0 + HALF], op=OP.add
                    )
                    z_t = zpool.tile([128, HALF], BF16, tag="z")
                    nc.scalar.activation(
                        z_t[:],
                        w_t[:],
                        AF.Prelu,
                        bias=cjT[:, 2 * jt : 2 * jt + 1],
                        alpha=SLOPE,
                    )
                    e_t = epool.tile([128, HALF], I16, tag="e")
                    nc.vector.tensor_scalar(
                        e_t[:], z_t[:], A_EXP, B_EXP, op0=OP.mult, op1=OP.add
                    )
                    e_bf = e_t[:].bitcast(BF16)
                    hr = h_sb[:, jt * 128 : (jt + 1) * 128]
                    for c in range(HALF // 512):
                        nc.tensor.matmul(
                            yT_ps[:, c * 512 : (c + 1) * 512],
                            hr,
                            e_bf[:, c * 512 : (c + 1) * 512],
                            start=(jt == 0),
                            stop=(jt == NJT - 1),
                        )
                    for c in range(HALF // 512):
                        nc.tensor.matmul(
                            rs_ps[0:1, c * 512 : (c + 1) * 512],
                            ones_sb[:],
                            e_bf[:, c * 512 : (c + 1) * 512],
                            start=(jt == 0),
                            stop=(jt == NJT - 1),
                        )

                yT_sb = fin.tile([128, HALF], BF16, tag="yT_sb")
                nc.scalar.copy(yT_sb[:], yT_ps[:])
                rs_sb = fin.tile([1, HALF], F32, tag="rs_sb")
                nc.vector.tensor_copy(rs_sb[:], rs_ps[:])

            recip_row = fin.tile([1, HALF], BF16, tag="recip_row")
            with nc.allow_low_precision(reason="recip feeds bf16 normalize"):
                nc.vector.reciprocal(recip_row[:], rs_sb[:])
            recipB = fin.tile([128, HALF], BF16, tag="recipB")
            for c in range(4):
                nc.gpsimd.partition_broadcast(
                    recipB[:, c * 512 : (c + 1) * 512],
                    recip_row[0:1, c * 512 : (c + 1) * 512],
                )
            ytn = fin.tile([128, HALF], BF16, tag="ytn")
            nc.vector.tensor_tensor(ytn[:], yT_sb[:], recipB[:], op=OP.mult)
            nc.vector.tensor_tensor(
                ytn[:], ytn[:], rT_sb[:, i0 : i0 + HALF], op=OP.add
            )
            nc.sync.dma_start(y_d[:, i0 : i0 + HALF], ytn[:])

    nc.compile()
    return nc


def _get_program():
    global _prog
    if _prog is None:
        _prog = build_program()
    return _prog


def _prepare_in_maps(x, graph, W, w_i, w_j, W_r):
    bf = ml_dtypes.bfloat16
    xT = np.ascontiguousarray(x.T).astype(bf)
    mask = np.where(graph > 0, np.float32(0.0), np.float32(MASK_NEG)).astype(bf)
    eye = np.eye(128, dtype=np.float32).astype(bf)
    ones = np.ones((128, 1), dtype=np.float32).astype(bf)
    in_maps = []
    for c in range(HEADS):
        wj2 = np.zeros((HF, 2), dtype=np.float32)
        wj2[:, 0] = np.asarray(w_j[c], dtype=np.float32).reshape(HF)
        in_maps.append(
            {
                "xT": xT,
                "mask": mask,
                "W": np.ascontiguousarray(W[c]).astype(bf),
                "Wr": np.ascontiguousarray(W_r[:, c * HF : (c + 1) * HF]).astype(bf),
                "wi": np.asarray(w_i[c], dtype=np.float32).astype(bf),
                "wj2": wj2.astype(bf),
                "ones": ones,
                "eye": eye,
            }
        )
    return in_maps


def run(inputs, trace=False, **kwargs):
    """Run the SPMD kernel; returns (y_full, BassKernelResults)."""
    x = np.asarray(inputs["x"], dtype=np.float32)
    graph = np.asarray(inputs["graph"])
    W = np.asarray(inputs["W"], dtype=np.float32)
    w_i = np.asarray(inputs["w_i"], dtype=np.float32)
    w_j = np.asarray(inputs["w_j"], dtype=np.float32)
    W_r = np.asarray(inputs["W_r"], dtype=np.float32)
    bias = np.asarray(inputs["bias"], dtype=np.float32)

    nc = _get_program()
    in_maps = _prepare_in_maps(x, graph, W, w_i, w_j, W_r)
    br = run_bass_kernel_spmd(
        nc, in_maps, core_ids=list(range(HEADS)), trace=trace, **kwargs
    )
    y = np.concatenate(
        [br.results[c]["y"].astype(np.float32).T for c in range(HEADS)], axis=1
    )
    y = y + bias[None, :]
    return y.astype(np.float32), br


def kernel(**inputs):
    y, _ = run(inputs)
    return y


# revision 18
# speedup vs baseline: 1.7347x; 1.3182x over previous
"""GAT forward on 8 Trainium2 NeuronCores — one attention head per core.

Math (per head, all [4096] nodes):
    h    = x @ W                    [N, 128]
    ci   = h @ w_i  (per-node)      [N]
    cj   = h @ w_j  (per-node)      [N]
    z    = leaky_relu(ci[i] + cj[j] + m[j,i])   (m = 0 / -60 additive mask)
    e    = exp(z)  (Schraudolph bit-trick on DVE, bf16)
    yT[f,i] = sum_j h[j,f] * e[j,i]            (PE matmul, e moving)
    rs[i]   = sum_j e[j,i]                     (PE matmul vs ones4 stationary)
    out[f,i] = yT[f,i]/rs[i] + residT[f,i];  host transposes to [i,f].

Engine assignment per attention tile [128j x 2048i] (64 tiles):
  - DVE:  w = mask + ciB          (tensor_tensor bf16, 2x mode)
          e = int16(z*A + B)      (tensor_scalar bf16->int16, Schraudolph exp;
                                   bitcast to bf16 feeds the PE directly)
  - ACT:  z = Prelu(w + cj_bias)  (one op does the cj add AND the leaky relu)
  - PE:   4x yT matmuls + 4x rowsum matmuls (all bf16, 512-col chunks)
  - Pool: mask DMA issue + ci/recip partition broadcasts
All matmuls everywhere are bf16 (1 cyc/row); phase 1 computes hT and residT
with the same streamed xT tiles. rowsum lands in one PSUM bank via a [4,512]
layout (block-diagonal ones stationary). Host does transpose/concat/bias.
"""
import sys

sys.path.insert(0, "/opt/trn_rl_repo")
from contextlib import ExitStack

import numpy as np
import ml_dtypes

import concourse.bass as bass
import concourse.tile as tile
from concourse import bacc, mybir
from concourse.bass_utils import run_bass_kernel_spmd

dt = mybir.dt
F32, BF16, I16 = dt.float32, dt.bfloat16, dt.int16
AF = mybir.ActivationFunctionType
OP = mybir.AluOpType

N = 4096
IN_F = 512
HF = 128
HEADS = 8
SLOPE = 0.2
MASK_NEG = -60.0
HALF = 2048
NJT = N // 128  # 32 j-tiles
NMC = IN_F // 128  # 4 contraction chunks
A_EXP = 184.6650292  # 128 * log2(e)
B_EXP = 16248.58  # 127*128 - schraudolph correction (round-to-nearest)

_prog = None


def build_program():
    nc = bacc.Bacc("TRN2", target_bir_lowering=False, debug=False)
    xT_d = nc.dram_tensor("xT", [IN_F, N], BF16, kind="ExternalInput").ap()
    mask_d = nc.dram_tensor("mask", [N, N], BF16, kind="ExternalInput").ap()
    W_d = nc.dram_tensor("W", [IN_F, HF], BF16, kind="ExternalInput").ap()
    Wr_d = nc.dram_tensor("Wr", [IN_F, HF], BF16, kind="ExternalInput").ap()
    wi_d = nc.dram_tensor("wi", [HF, 1], BF16, kind="ExternalInput").ap()
    wj2_d = nc.dram_tensor("wj2", [HF, 2], BF16, kind="ExternalInput").ap()
    ones_d = nc.dram_tensor("ones", [128, 1], BF16, kind="ExternalInput").ap()
    eye_d = nc.dram_tensor("eye", [128, 128], BF16, kind="ExternalInput").ap()
    y_d = nc.dram_tensor("y", [HF, N], BF16, kind="ExternalOutput").ap()

    with tile.TileContext(nc) as tc, ExitStack() as ctx:
        persist = ctx.enter_context(tc.tile_pool(name="persist", bufs=1))
        h_sb = persist.tile([128, N], BF16, tag="h")  # h[j,f] per j-tile
        rT_sb = persist.tile([128, N], BF16, tag="rT")  # residT[f,i]
        ciB = persist.tile([128, N], BF16, tag="ciB")  # ci bcast along partitions
        cjT = persist.tile([128, 2 * NJT], F32, tag="cjT")  # cj at even cols
        ones_sb = persist.tile([128, 1], BF16, tag="ones")
        eye_sb = persist.tile([128, 128], BF16, tag="eye")
        nc.gpsimd.dma_start(ones_sb[:], ones_d)
        nc.gpsimd.dma_start(eye_sb[:], eye_d)

        # Phase-2 pools opened first so their SBUF is disjoint from phase-1
        # scoped buffers.
        mpool = ctx.enter_context(tc.tile_pool(name="mpool", bufs=8))
        wpool = ctx.enter_context(tc.tile_pool(name="wpool", bufs=4))
        zpool = ctx.enter_context(tc.tile_pool(name="zpool", bufs=4))
        epool = ctx.enter_context(tc.tile_pool(name="epool", bufs=4))
        fin = ctx.enter_context(tc.tile_pool(name="fin", bufs=2))

        # ---------- Phase 1: hT/residT over streamed xT; ci/cj/h ----------
        with ExitStack() as p1:
            ph1 = p1.enter_context(tc.tile_pool(name="ph1", bufs=1))
            xpool = p1.enter_context(tc.tile_pool(name="xpool", bufs=3))
            hTp = p1.enter_context(tc.tile_pool(name="hTp", bufs=2))

            # First x tile goes out first so the PE starts ASAP; small
            # constants ride the gpsimd queue.
            xt0 = xpool.tile([128, HALF], BF16, tag="xt")
            nc.sync.dma_start(xt0[:], xT_d[0:128, 0:HALF])
            W_sb = ph1.tile([128, NMC * HF], BF16, tag="W")
            Wr_sb = ph1.tile([128, NMC * HF], BF16, tag="Wr")
            for mc in range(NMC):
                nc.sync.dma_start(
                    W_sb[:, mc * HF : (mc + 1) * HF], W_d[mc * 128 : (mc + 1) * 128, :]
                )
                nc.sync.dma_start(
                    Wr_sb[:, mc * HF : (mc + 1) * HF],
                    Wr_d[mc * 128 : (mc + 1) * 128, :],
                )
            wi_sb = ph1.tile([128, 1], BF16, tag="wi")
            nc.gpsimd.dma_start(wi_sb[:], wi_d)
            wj2_sb = ph1.tile([128, 2], BF16, tag="wj2")
            nc.gpsimd.dma_start(wj2_sb[:], wj2_d)

            for hf in range(2):
                o = hf * HALF
                hT_sb = hTp.tile([128, HALF], BF16, tag="hT")
                with ExitStack() as ps1:
                    psA = ps1.enter_context(
                        tc.tile_pool(name=f"psA{hf}", bufs=1, space="PSUM")
                    )
                    ps_hT = psA.tile([128, HALF], F32, tag="ps_hT")
                    ps_rT = psA.tile([128, HALF], F32, tag="ps_rT")
                    for mc in range(NMC):
                        if hf == 0 and mc == 0:
                            xt = xt0
                        else:
                            xt = xpool.tile([128, HALF], BF16, tag="xt")
                            nc.sync.dma_start(
                                xt[:], xT_d[mc * 128 : (mc + 1) * 128, o : o + HALF]
                            )
                        for ck in range(HALF // 512):
                            nc.tensor.matmul(
                                ps_hT[:, ck * 512 : (ck + 1) * 512],
                                W_sb[:, mc * HF : (mc + 1) * HF],
                                xt[:, ck * 512 : (ck + 1) * 512],
                                start=(mc == 0),
                                stop=(mc == NMC - 1),
                            )
                        for ck in range(HALF // 512):
                            nc.tensor.matmul(
                                ps_rT[:, ck * 512 : (ck + 1) * 512],
                                Wr_sb[:, mc * HF : (mc + 1) * HF],
                                xt[:, ck * 512 : (ck + 1) * 512],
                                start=(mc == 0),
                                stop=(mc == NMC - 1),
                            )
                    nc.vector.tensor_copy(hT_sb[:], ps_hT[:])
                    nc.scalar.copy(rT_sb[:, o : o + HALF], ps_rT[:])

                with ExitStack() as ps2:
                    psB = ps2.enter_context(
                        tc.tile_pool(name=f"psB{hf}", bufs=1, space="PSUM")
                    )
                    ps_ci = psB.tile([1, HALF], F32, tag="ps_ci")
                    for c in range(4):
                        nc.tensor.matmul(
                            ps_ci[0:1, c * 512 : (c + 1) * 512],
                            wi_sb[:],
                            hT_sb[:, c * 512 : (c + 1) * 512],
                            start=True,
                            stop=True,
                        )
                    ci_row = ph1.tile([1, HALF], BF16, tag=f"ci_row{hf}")
                    nc.vector.tensor_copy(ci_row[:], ps_ci[:])
                    for c in range(4):
                        nc.gpsimd.partition_broadcast(
                            ciB[:, o + c * 512 : o + (c + 1) * 512],
                            ci_row[0:1, c * 512 : (c + 1) * 512],
                        )

                    ps_cj = psB.tile([128, NJT], F32, tag="ps_cj")
                    for k in range(NJT // 2):
                        jt = hf * (NJT // 2) + k
                        nc.tensor.matmul(
                            ps_cj[:, 2 * k : 2 * k + 2],
                            hT_sb[:, k * 128 : (k + 1) * 128],
                            wj2_sb[:],
                            start=(k == 0),
                            stop=(k == NJT // 2 - 1),
                        )
                    nc.vector.tensor_copy(cjT[:, hf * NJT : (hf + 1) * NJT], ps_cj[:])

                    ps_h = psB.tile([128, HALF], BF16, tag="ps_h")
                    for k in range(HALF // 128):
                        nc.tensor.transpose(
                            ps_h[:, k * 128 : (k + 1) * 128],
                            hT_sb[:, k * 128 : (k + 1) * 128],
                            eye_sb[:],
                        )
                    nc.scalar.copy(h_sb[:, o : o + HALF], ps_h[:])

        # ---------- Phase 2: attention ----------
        for half in range(2):
            i0 = half * HALF
            with ExitStack() as pmm_ctx:
                pmm = pmm_ctx.enter_context(
                    tc.tile_pool(name=f"pmm{half}", bufs=1, space="PSUM")
                )
                yT_ps = pmm.tile([128, HALF], F32, tag="yT")
                rs_ps = pmm.tile([1, HALF], F32, tag="rs")

                for jt in range(NJT):
                    m_t = mpool.tile([128, HALF], BF16, tag="m")
                    nc.gpsimd.dma_start(
                        m_t[:], mask_d[jt * 128 : (jt + 1) * 128, i0 : i0 + HALF]
                    )
                    w_t = wpool.tile([128, HALF], BF16, tag="w")
                    nc.vector.tensor_tensor(
                        w_t[:], m_t[:], ciB[:, i0 : i0 + HALF], op=OP.add
                    )
                    z_t = zpool.tile([128, HALF], BF16, tag="z")
                    nc.scalar.activation(
                        z_t[:],
                        w_t[:],
                        AF.Prelu,
                        bias=cjT[:, 2 * jt : 2 * jt + 1],
                        alpha=SLOPE,
                    )
                    e_t = epool.tile([128, HALF], I16, tag="e")
                    nc.vector.tensor_scalar(
                        e_t[:], z_t[:], A_EXP, B_EXP, op0=OP.mult, op1=OP.add
                    )
                    e_bf = e_t[:].bitcast(BF16)
                    hr = h_sb[:, jt * 128 : (jt + 1) * 128]
                    for c in range(HALF // 512):
                        nc.tensor.matmul(
                            yT_ps[:, c * 512 : (c + 1) * 512],
                            hr,
                            e_bf[:, c * 512 : (c + 1) * 512],
                            start=(jt == 0),
                            stop=(jt == NJT - 1),
                        )
                    for c in range(HALF // 512):
                        nc.tensor.matmul(
                            rs_ps[0:1, c * 512 : (c + 1) * 512],
                            ones_sb[:],
                            e_bf[:, c * 512 : (c + 1) * 512],
                            start=(jt == 0),
                            stop=(jt == NJT - 1),
                        )

                # Finale: approx-recip the rowsums, broadcast, normalize, DMA.
                yT_sb = fin.tile([128, HALF], BF16, tag="yT_sb")
                nc.scalar.copy(yT_sb[:], yT_ps[:])
                recip_row = fin.tile([1, HALF], F32, tag="recip_row")
                nc.vector.reciprocal_approx_fast(recip_row[:], rs_ps[0:1, :])
                recipB = fin.tile([128, HALF], F32, tag="recipB")
                for c in range(4):
                    sl = slice(c * 512, (c + 1) * 512)
                    nc.gpsimd.partition_broadcast(
                        recipB[:, sl], recip_row[0:1, sl]
                    )
                ytn = fin.tile([128, HALF], BF16, tag="ytn")
                for c in range(4):
                    sl = slice(c * 512, (c + 1) * 512)
                    nc.vector.tensor_tensor(
                        ytn[:, sl], yT_sb[:, sl], recipB[:, sl], op=OP.mult
                    )
                    nc.vector.tensor_tensor(
                        ytn[:, sl],
                        ytn[:, sl],
                        rT_sb[:, i0 + c * 512 : i0 + (c + 1) * 512],
                        op=OP.add,
                    )
                    nc.sync.dma_start(
                        y_d[:, i0 + c * 512 : i0 + (c + 1) * 512], ytn[:, sl]
                    )

    nc.compile()
    return nc


def _get_program():
    global _prog
    if _prog is None:
        _prog = build_program()
    return _prog


def _prepare_in_maps(x, graph, W, w_i, w_j, W_r):
    bf = ml_dtypes.bfloat16
    xT = np.ascontiguousarray(x.T).astype(bf)
    mask = np.where(graph > 0, np.float32(0.0), np.float32(MASK_NEG)).astype(bf)
    eye = np.eye(128, dtype=np.float32).astype(bf)
    ones = np.ones((128, 1), dtype=np.float32).astype(bf)
    in_maps = []
    for c in range(HEADS):
        wj2 = np.zeros((HF, 2), dtype=np.float32)
        wj2[:, 0] = np.asarray(w_j[c], dtype=np.float32).reshape(HF)
        in_maps.append(
            {
                "xT": xT,
                "mask": mask,
                "W": np.ascontiguousarray(W[c]).astype(bf),
                "Wr": np.ascontiguousarray(W_r[:, c * HF : (c + 1) * HF]).astype(bf),
                "wi": np.asarray(w_i[c], dtype=np.float32).astype(bf),
                "wj2": wj2.astype(bf),
                "ones": ones,
                "eye": eye,
            }
        )
    return in_maps


def run(inputs, trace=False, **kwargs):
    """Run the SPMD kernel; returns (y_full, BassKernelResults)."""
    x = np.asarray(inputs["x"], dtype=np.float32)
    graph = np.asarray(inputs["graph"])
    W = np.asarray(inputs["W"], dtype=np.float32)
    w_i = np.asarray(inputs["w_i"], dtype=np.float32)
    w_j = np.asarray(inputs["w_j"], dtype=np.float32)
    W_r = np.asarray(inputs["W_r"], dtype=np.float32)
    bias = np.asarray(inputs["bias"], dtype=np.float32)

    nc = _get_program()
    in_maps = _prepare_in_maps(x, graph, W, w_i, w_j, W_r)
    br = run_bass_kernel_spmd(
        nc, in_maps, core_ids=list(range(HEADS)), trace=trace, **kwargs
    )
    y = np.concatenate(
        [br.results[c]["y"].astype(np.float32).T for c in range(HEADS)], axis=1
    )
    y = y + bias[None, :]
    return y.astype(np.float32), br


def kernel(**inputs):
    y, _ = run(inputs)
    return y


# revision 25
# speedup vs baseline: 1.7356x; 1.0005x over previous
"""GAT forward on 8 Trainium2 NeuronCores — one attention head per core.

Math (per head, all [4096] nodes):
    h    = x @ W                    [N, 128]
    ci   = h @ w_i  (per-node)      [N]
    cj   = h @ w_j  (per-node)      [N]
    z    = leaky_relu(ci[i] + cj[j] + m[j,i])   (m = 0 / -60 additive mask)
    e    = exp(z)  (Schraudolph bit-trick on DVE, bf16)
    yT[f,i] = sum_j h[j,f] * e[j,i]            (PE matmul, e moving)
    rs[i]   = sum_j e[j,i]                     (PE matmul vs ones4 stationary)
    out[f,i] = yT[f,i]/rs[i] + residT[f,i];  host transposes to [i,f].

Engine assignment per attention tile [128j x 2048i] (64 tiles):
  - DVE:  w = mask + ciB          (tensor_tensor bf16, 2x mode)
          e = int16(z*A + B)      (tensor_scalar bf16->int16, Schraudolph exp;
                                   bitcast to bf16 feeds the PE directly)
  - ACT:  z = Prelu(w + cj_bias)  (one op does the cj add AND the leaky relu)
  - PE:   4x yT matmuls + 4x rowsum matmuls (all bf16, 512-col chunks)
  - Pool: mask DMA issue + ci/recip partition broadcasts
All matmuls everywhere are bf16 (1 cyc/row); phase 1 computes hT and residT
with the same streamed xT tiles. rowsum lands in one PSUM bank via a [4,512]
layout (block-diagonal ones stationary). Host does transpose/concat/bias.
"""
import sys

sys.path.insert(0, "/opt/trn_rl_repo")
from contextlib import ExitStack

import numpy as np
import ml_dtypes

import concourse.bass as bass
import concourse.tile as tile
from concourse import bacc, mybir
from concourse.bass_utils import run_bass_kernel_spmd

dt = mybir.dt
F32, BF16, I16 = dt.float32, dt.bfloat16, dt.int16
AF = mybir.ActivationFunctionType
OP = mybir.AluOpType

N = 4096
IN_F = 512
HF = 128
HEADS = 8
SLOPE = 0.2
MASK_NEG = -60.0
HALF = 2048
NJT = N // 128  # 32 j-tiles
NMC = IN_F // 128  # 4 contraction chunks
A_EXP = 184.6650292  # 128 * log2(e)
B_EXP = 16248.58  # 127*128 - schraudolph correction (round-to-nearest)

_prog = None


def build_program():
    nc = bacc.Bacc("TRN2", target_bir_lowering=False, debug=False)
    xT_d = nc.dram_tensor("xT", [IN_F, N], BF16, kind="ExternalInput").ap()
    mask_d = nc.dram_tensor("mask", [N, N], BF16, kind="ExternalInput").ap()
    W_d = nc.dram_tensor("W", [IN_F, HF], BF16, kind="ExternalInput").ap()
    Wr_d = nc.dram_tensor("Wr", [IN_F, HF], BF16, kind="ExternalInput").ap()
    Wwi_d = nc.dram_tensor("Wwi", [IN_F, 1], BF16, kind="ExternalInput").ap()
    wj2_d = nc.dram_tensor("wj2", [HF, 2], BF16, kind="ExternalInput").ap()
    ones_d = nc.dram_tensor("ones", [128, 1], BF16, kind="ExternalInput").ap()
    eye_d = nc.dram_tensor("eye", [128, 128], BF16, kind="ExternalInput").ap()
    y_d = nc.dram_tensor("y", [HF, N], BF16, kind="ExternalOutput").ap()

    with tile.TileContext(nc) as tc, ExitStack() as ctx:
        persist = ctx.enter_context(tc.tile_pool(name="persist", bufs=1))
        h_sb = persist.tile([128, N], BF16, tag="h")  # h[j,f] per j-tile
        rT_sb = persist.tile([128, N], BF16, tag="rT")  # residT[f,i]
        ciB = persist.tile([128, N], BF16, tag="ciB")  # ci bcast along partitions
        cjT = persist.tile([128, 2 * NJT], F32, tag="cjT")  # cj at even cols
        ones_sb = persist.tile([128, 1], BF16, tag="ones")
        eye_sb = persist.tile([128, 128], BF16, tag="eye")
        nc.gpsimd.dma_start(ones_sb[:], ones_d)
        nc.gpsimd.dma_start(eye_sb[:], eye_d)

        # Phase-2 pools opened first so their SBUF is disjoint from phase-1
        # scoped buffers.
        mpool = ctx.enter_context(tc.tile_pool(name="mpool", bufs=8))
        wpool = ctx.enter_context(tc.tile_pool(name="wpool", bufs=4))
        zpool = ctx.enter_context(tc.tile_pool(name="zpool", bufs=4))
        epool = ctx.enter_context(tc.tile_pool(name="epool", bufs=4))
        fin = ctx.enter_context(tc.tile_pool(name="fin", bufs=2))

        # ---------- Phase 1: hT/residT over streamed xT; ci/cj/h ----------
        with ExitStack() as p1:
            ph1 = p1.enter_context(tc.tile_pool(name="ph1", bufs=1))
            xpool = p1.enter_context(tc.tile_pool(name="xpool", bufs=5))
            hTp = p1.enter_context(tc.tile_pool(name="hTp", bufs=2))

            # First x tile goes out first so the PE starts ASAP; small
            # constants ride the gpsimd queue.
            xt0 = xpool.tile([128, HALF], BF16, tag="xt")
            nc.sync.dma_start(xt0[:], xT_d[0:128, 0:HALF])
            W_sb = ph1.tile([128, NMC * HF], BF16, tag="W")
            Wr_sb = ph1.tile([128, NMC * HF], BF16, tag="Wr")
            for mc in range(NMC):
                nc.sync.dma_start(
                    W_sb[:, mc * HF : (mc + 1) * HF], W_d[mc * 128 : (mc + 1) * 128, :]
                )
                nc.sync.dma_start(
                    Wr_sb[:, mc * HF : (mc + 1) * HF],
                    Wr_d[mc * 128 : (mc + 1) * 128, :],
                )
            Wwi_sb = ph1.tile([128, NMC], BF16, tag="Wwi")
            for mc in range(NMC):
                nc.gpsimd.dma_start(
                    Wwi_sb[:, mc : mc + 1], Wwi_d[mc * 128 : (mc + 1) * 128, :]
                )
            wj2_sb = ph1.tile([128, 2], BF16, tag="wj2")
            nc.gpsimd.dma_start(wj2_sb[:], wj2_d)

            for hf in range(2):
                o = hf * HALF
                hT_sb = hTp.tile([128, HALF], BF16, tag="hT")
                xts = []
                # Loop 1: hT (+ in-stream ci via host-precomputed W@w_i) so
                # ciB is ready early and phase-2 elementwise can start while
                # the rest of phase 1 still owns the PE.
                with ExitStack() as ps1:
                    psA = ps1.enter_context(
                        tc.tile_pool(name=f"psA{hf}", bufs=1, space="PSUM")
                    )
                    ps_hT = psA.tile([128, HALF], F32, tag="ps_hT")
                    ps_ci = psA.tile([1, HALF], F32, tag="ps_ci")
                    for mc in range(NMC):
                        if hf == 0 and mc == 0:
                            xt = xt0
                        else:
                            xt = xpool.tile([128, HALF], BF16, tag="xt")
                            nc.sync.dma_start(
                                xt[:], xT_d[mc * 128 : (mc + 1) * 128, o : o + HALF]
                            )
                        xts.append(xt)
                        for ck in range(HALF // 512):
                            nc.tensor.matmul(
                                ps_hT[:, ck * 512 : (ck + 1) * 512],
                                W_sb[:, mc * HF : (mc + 1) * HF],
                                xt[:, ck * 512 : (ck + 1) * 512],
                                start=(mc == 0),
                                stop=(mc == NMC - 1),
                            )
                        for ck in range(HALF // 512):
                            nc.tensor.matmul(
                                ps_ci[0:1, ck * 512 : (ck + 1) * 512],
                                Wwi_sb[:, mc : mc + 1],
                                xt[:, ck * 512 : (ck + 1) * 512],
                                start=(mc == 0),
                                stop=(mc == NMC - 1),
                            )
                    nc.vector.tensor_copy(hT_sb[:], ps_hT[:])
                    ci_row = ph1.tile([1, HALF], BF16, tag=f"ci_row{hf}")
                    nc.vector.tensor_copy(ci_row[:], ps_ci[:])
                    for c in range(4):
                        nc.gpsimd.partition_broadcast(
                            ciB[:, o + c * 512 : o + (c + 1) * 512],
                            ci_row[0:1, c * 512 : (c + 1) * 512],
                        )

                # Loop 2: residT (reusing the resident x tiles), cj, h.
                with ExitStack() as ps2:
                    psB = ps2.enter_context(
                        tc.tile_pool(name=f"psB{hf}", bufs=1, space="PSUM")
                    )
                    ps_rT = psB.tile([128, HALF], F32, tag="ps_rT")
                    for mc in range(NMC):
                        for ck in range(HALF // 512):
                            nc.tensor.matmul(
                                ps_rT[:, ck * 512 : (ck + 1) * 512],
                                Wr_sb[:, mc * HF : (mc + 1) * HF],
                                xts[mc][:, ck * 512 : (ck + 1) * 512],
                                start=(mc == 0),
                                stop=(mc == NMC - 1),
                            )
                    nc.scalar.copy(rT_sb[:, o : o + HALF], ps_rT[:])

                    ps_cj = psB.tile([128, NJT], F32, tag="ps_cj")
                    for k in range(NJT // 2):
                        nc.tensor.matmul(
                            ps_cj[:, 2 * k : 2 * k + 2],
                            hT_sb[:, k * 128 : (k + 1) * 128],
                            wj2_sb[:],
                            start=(k == 0),
                            stop=(k == NJT // 2 - 1),
                        )
                    nc.vector.tensor_copy(cjT[:, hf * NJT : (hf + 1) * NJT], ps_cj[:])

                    ps_h = psB.tile([128, HALF], BF16, tag="ps_h")
                    for k in range(HALF // 128):
                        nc.tensor.transpose(
                            ps_h[:, k * 128 : (k + 1) * 128],
                            hT_sb[:, k * 128 : (k + 1) * 128],
                            eye_sb[:],
                        )
                    nc.scalar.copy(h_sb[:, o : o + HALF], ps_h[:])

        # ---------- Phase 2: attention ----------
        for half in range(2):
            i0 = half * HALF
            with ExitStack() as pmm_ctx:
                pmm = pmm_ctx.enter_context(
                    tc.tile_pool(name=f"pmm{half}", bufs=1, space="PSUM")
                )
                yT_ps = pmm.tile([128, HALF], F32, tag="yT")
                rs_ps = pmm.tile([1, HALF], F32, tag="rs")

                for jt in range(NJT):
                    m_t = mpool.tile([128, HALF], BF16, tag="m")
                    # Alternate DMA queues so transfers and end-of-program
                    # queue drains parallelize.
                    dma_eng = nc.gpsimd if jt % 2 == 0 else nc.sync
                    dma_eng.dma_start(
                        m_t[:], mask_d[jt * 128 : (jt + 1) * 128, i0 : i0 + HALF]
                    )
                    w_t = wpool.tile([128, HALF], BF16, tag="w")
                    nc.vector.tensor_tensor(
                        w_t[:], m_t[:], ciB[:, i0 : i0 + HALF], op=OP.add
                    )
                    z_t = zpool.tile([128, HALF], BF16, tag="z")
                    nc.scalar.activation(
                        z_t[:],
                        w_t[:],
                        AF.Prelu,
                        bias=cjT[:, 2 * jt : 2 * jt + 1],
                        alpha=SLOPE,
                    )
                    e_t = epool.tile([128, HALF], I16, tag="e")
                    nc.vector.tensor_scalar(
                        e_t[:], z_t[:], A_EXP, B_EXP, op0=OP.mult, op1=OP.add
                    )
                    e_bf = e_t[:].bitcast(BF16)
                    hr = h_sb[:, jt * 128 : (jt + 1) * 128]
                    # rs first so the final rowsum (tail-critical for the
                    # reciprocal) completes before the last yT matmuls.
                    for c in range(HALF // 512):
                        nc.tensor.matmul(
                            rs_ps[0:1, c * 512 : (c + 1) * 512],
                            ones_sb[:],
                            e_bf[:, c * 512 : (c + 1) * 512],
                            start=(jt == 0),
                            stop=(jt == NJT - 1),
                        )
                    for c in range(HALF // 512):
                        nc.tensor.matmul(
                            yT_ps[:, c * 512 : (c + 1) * 512],
                            hr,
                            e_bf[:, c * 512 : (c + 1) * 512],
                            start=(jt == 0),
                            stop=(jt == NJT - 1),
                        )

                # Finale: approx-recip the rowsums, broadcast, normalize, DMA.
                yT_sb = fin.tile([128, HALF], BF16, tag="yT_sb")
                nc.scalar.copy(yT_sb[:], yT_ps[:])
                recip_row = fin.tile([1, HALF], F32, tag="recip_row")
                nc.vector.reciprocal_approx_fast(recip_row[:], rs_ps[0:1, :])
                recipB = fin.tile([128, HALF], F32, tag="recipB")
                for c in range(4):
                    sl = slice(c * 512, (c + 1) * 512)
                    nc.gpsimd.partition_broadcast(
                        recipB[:, sl], recip_row[0:1, sl]
                    )
                ytn = fin.tile([128, HALF], BF16, tag="ytn")
                for c in range(4):
                    sl = slice(c * 512, (c + 1) * 512)
                    nc.vector.tensor_tensor(
                        ytn[:, sl], yT_sb[:, sl], recipB[:, sl], op=OP.mult
                    )
                    nc.vector.tensor_tensor(
                        ytn[:, sl],
                        ytn[:, sl],
                        rT_sb[:, i0 + c * 512 : i0 + (c + 1) * 512],
                        op=OP.add,
                    )
                    nc.scalar.dma_start(
                        y_d[:, i0 + c * 512 : i0 + (c + 1) * 512], ytn[:, sl]
                    )

    nc.compile()
    return nc


def _get_program():
    global _prog
    if _prog is None:
        _prog = build_program()
    return _prog


def _prepare_in_maps(x, graph, W, w_i, w_j, W_r):
    bf = ml_dtypes.bfloat16
    xT = np.ascontiguousarray(x.T).astype(bf)
    mask = np.where(graph > 0, np.float32(0.0), np.float32(MASK_NEG)).astype(bf)
    eye = np.eye(128, dtype=np.float32).astype(bf)
    ones = np.ones((128, 1), dtype=np.float32).astype(bf)
    in_maps = []
    for c in range(HEADS):
        wj2 = np.zeros((HF, 2), dtype=np.float32)
        wj2[:, 0] = np.asarray(w_j[c], dtype=np.float32).reshape(HF)
        in_maps.append(
            {
                "xT": xT,
                "mask": mask,
                "W": np.ascontiguousarray(W[c]).astype(bf),
                "Wr": np.ascontiguousarray(W_r[:, c * HF : (c + 1) * HF]).astype(bf),
                "Wwi": (
                    np.asarray(W[c], dtype=np.float32)
                    @ np.asarray(w_i[c], dtype=np.float32)
                ).astype(bf),
                "wj2": wj2.astype(bf),
                "ones": ones,
                "eye": eye,
            }
        )
    return in_maps


def run(inputs, trace=False, **kwargs):
    """Run the SPMD kernel; returns (y_full, BassKernelResults)."""
    x = np.asarray(inputs["x"], dtype=np.float32)
    graph = np.asarray(inputs["graph"])
    W = np.asarray(inputs["W"], dtype=np.float32)
    w_i = np.asarray(inputs["w_i"], dtype=np.float32)
    w_j = np.asarray(inputs["w_j"], dtype=np.float32)
    W_r = np.asarray(inputs["W_r"], dtype=np.float32)
    bias = np.asarray(inputs["bias"], dtype=np.float32)

    nc = _get_program()
    in_maps = _prepare_in_maps(x, graph, W, w_i, w_j, W_r)
    br = run_bass_kernel_spmd(
        nc, in_maps, core_ids=list(range(HEADS)), trace=trace, **kwargs
    )
    y = np.concatenate(
        [br.results[c]["y"].astype(np.float32).T for c in range(HEADS)], axis=1
    )
    y = y + bias[None, :]
    return y.astype(np.float32), br


def kernel(**inputs):
    y, _ = run(inputs)
    return y


# revision 28
# speedup vs baseline: 1.7757x; 1.0231x over previous
"""GAT forward on 8 Trainium2 NeuronCores — one attention head per core.

Math (per head, all [4096] nodes):
    h    = x @ W                    [N, 128]
    ci   = h @ w_i  (per-node)      [N]
    cj   = h @ w_j  (per-node)      [N]
    z    = leaky_relu(ci[i] + cj[j] + m[j,i])   (m = 0 / -60 additive mask)
    e    = exp(z)  (Schraudolph bit-trick on DVE, bf16)
    yT[f,i] = sum_j h[j,f] * e[j,i]            (PE matmul, e moving)
    rs[i]   = sum_j e[j,i]                     (PE matmul vs ones4 stationary)
    out[f,i] = yT[f,i]/rs[i] + residT[f,i];  host transposes to [i,f].

Engine assignment per attention tile [128j x 2048i] (64 tiles):
  - DVE:  w = mask + ciB          (tensor_tensor bf16, 2x mode)
          e = int16(z*A + B)      (tensor_scalar bf16->int16, Schraudolph exp;
                                   bitcast to bf16 feeds the PE directly)
  - ACT:  z = Prelu(w + cj_bias)  (one op does the cj add AND the leaky relu)
  - PE:   4x yT matmuls + 4x rowsum matmuls (all bf16, 512-col chunks)
  - Pool: mask DMA issue + ci/recip partition broadcasts
All matmuls everywhere are bf16 (1 cyc/row); phase 1 computes hT and residT
with the same streamed xT tiles. rowsum lands in one PSUM bank via a [4,512]
layout (block-diagonal ones stationary). Host does transpose/concat/bias.
"""
import sys

sys.path.insert(0, "/opt/trn_rl_repo")
from contextlib import ExitStack

import numpy as np
import ml_dtypes

import concourse.bass as bass
import concourse.tile as tile
from concourse import bacc, mybir
from concourse.bass_utils import run_bass_kernel_spmd

dt = mybir.dt
F32, BF16, I16 = dt.float32, dt.bfloat16, dt.int16
AF = mybir.ActivationFunctionType
OP = mybir.AluOpType

N = 4096
IN_F = 512
HF = 128
HEADS = 8
SLOPE = 0.2
MASK_NEG = -60.0
HALF = 2048
NJT = N // 128  # 32 j-tiles
NMC = IN_F // 128  # 4 contraction chunks
A_EXP = 184.6650292  # 128 * log2(e)
B_EXP = 16248.58  # 127*128 - schraudolph correction (round-to-nearest)

_prog = None


def build_program():
    nc = bacc.Bacc("TRN2", target_bir_lowering=False, debug=False)
    xT_d = nc.dram_tensor("xT", [IN_F, N], BF16, kind="ExternalInput").ap()
    mask_d = nc.dram_tensor("mask", [N, N], BF16, kind="ExternalInput").ap()
    W_d = nc.dram_tensor("W", [IN_F, HF], BF16, kind="ExternalInput").ap()
    Wr_d = nc.dram_tensor("Wr", [IN_F, HF], BF16, kind="ExternalInput").ap()
    Wwi_d = nc.dram_tensor("Wwi", [IN_F, 1], BF16, kind="ExternalInput").ap()
    wj2_d = nc.dram_tensor("wj2", [HF, 2], BF16, kind="ExternalInput").ap()
    ones_d = nc.dram_tensor("ones", [128, 1], BF16, kind="ExternalInput").ap()
    eye_d = nc.dram_tensor("eye", [128, 128], BF16, kind="ExternalInput").ap()
    y_d = nc.dram_tensor("y", [HF, N], BF16, kind="ExternalOutput").ap()

    with tile.TileContext(nc) as tc, ExitStack() as ctx:
        persist = ctx.enter_context(tc.tile_pool(name="persist", bufs=1))
        h_sb = persist.tile([128, N], BF16, tag="h")  # h[j,f] per j-tile
        rT_sb = persist.tile([128, N], BF16, tag="rT")  # residT[f,i]
        ciB = persist.tile([128, N], BF16, tag="ciB")  # ci bcast along partitions
        cjT = persist.tile([128, 2 * NJT], F32, tag="cjT")  # cj at even cols
        ones_sb = persist.tile([128, 1], BF16, tag="ones")
        eye_sb = persist.tile([128, 128], BF16, tag="eye")
        nc.gpsimd.dma_start(ones_sb[:], ones_d)
        nc.gpsimd.dma_start(eye_sb[:], eye_d)

        # Phase-2 pools opened first so their SBUF is disjoint from phase-1
        # scoped buffers.
        mpool = ctx.enter_context(tc.tile_pool(name="mpool", bufs=8))
        wpool = ctx.enter_context(tc.tile_pool(name="wpool", bufs=4))
        zpool = ctx.enter_context(tc.tile_pool(name="zpool", bufs=4))
        epool = ctx.enter_context(tc.tile_pool(name="epool", bufs=4))
        fin = ctx.enter_context(tc.tile_pool(name="fin", bufs=2))

        # ---------- Phase 1: hT/residT over streamed xT; ci/cj/h ----------
        with ExitStack() as p1:
            ph1 = p1.enter_context(tc.tile_pool(name="ph1", bufs=1))
            xpool = p1.enter_context(tc.tile_pool(name="xpool", bufs=5))
            hTp = p1.enter_context(tc.tile_pool(name="hTp", bufs=2))

            # Sync queue: W chunks then x tiles back-to-back (PE-critical
            # path); everything else rides the gpsimd queue.
            W_sb = ph1.tile([128, NMC * HF], BF16, tag="W")
            Wr_sb = ph1.tile([128, NMC * HF], BF16, tag="Wr")
            Wwi_sb = ph1.tile([128, NMC], BF16, tag="Wwi")
            for mc in range(NMC):
                nc.sync.dma_start(
                    W_sb[:, mc * HF : (mc + 1) * HF], W_d[mc * 128 : (mc + 1) * 128, :]
                )
            xt0 = xpool.tile([128, HALF], BF16, tag="xt")
            nc.sync.dma_start(xt0[:], xT_d[0:128, 0:HALF])
            for mc in range(NMC):
                nc.gpsimd.dma_start(
                    Wwi_sb[:, mc : mc + 1], Wwi_d[mc * 128 : (mc + 1) * 128, :]
                )
                nc.gpsimd.dma_start(
                    Wr_sb[:, mc * HF : (mc + 1) * HF],
                    Wr_d[mc * 128 : (mc + 1) * 128, :],
                )
            wj2_sb = ph1.tile([128, 2], BF16, tag="wj2")
            nc.gpsimd.dma_start(wj2_sb[:], wj2_d)

            for hf in range(2):
                o = hf * HALF
                hT_sb = hTp.tile([128, HALF], BF16, tag="hT")
                xts = []
                # Loop 1: hT (+ in-stream ci via host-precomputed W@w_i) so
                # ciB is ready early and phase-2 elementwise can start while
                # the rest of phase 1 still owns the PE.
                with ExitStack() as ps1:
                    psA = ps1.enter_context(
                        tc.tile_pool(name=f"psA{hf}", bufs=1, space="PSUM")
                    )
                    ps_hT = psA.tile([128, HALF], F32, tag="ps_hT")
                    ps_ci = psA.tile([1, HALF], F32, tag="ps_ci")
                    for mc in range(NMC):
                        if hf == 0 and mc == 0:
                            xt = xt0
                        else:
                            xt = xpool.tile([128, HALF], BF16, tag="xt")
                            nc.sync.dma_start(
                                xt[:], xT_d[mc * 128 : (mc + 1) * 128, o : o + HALF]
                            )
                        xts.append(xt)
                        for ck in range(HALF // 512):
                            nc.tensor.matmul(
                                ps_hT[:, ck * 512 : (ck + 1) * 512],
                                W_sb[:, mc * HF : (mc + 1) * HF],
                                xt[:, ck * 512 : (ck + 1) * 512],
                                start=(mc == 0),
                                stop=(mc == NMC - 1),
                            )
                        for ck in range(HALF // 512):
                            nc.tensor.matmul(
                                ps_ci[0:1, ck * 512 : (ck + 1) * 512],
                                Wwi_sb[:, mc : mc + 1],
                                xt[:, ck * 512 : (ck + 1) * 512],
                                start=(mc == 0),
                                stop=(mc == NMC - 1),
                            )
                    # Evacuate on different engines so the PSUM frees fast.
                    nc.scalar.copy(hT_sb[:], ps_hT[:])
                    ci_row = ph1.tile([1, HALF], BF16, tag=f"ci_row{hf}")
                    nc.vector.tensor_copy(ci_row[:], ps_ci[:])
                    for c in range(4):
                        nc.gpsimd.partition_broadcast(
                            ciB[:, o + c * 512 : o + (c + 1) * 512],
                            ci_row[0:1, c * 512 : (c + 1) * 512],
                        )

                # Loop 2: residT (reusing the resident x tiles), cj, h.
                with ExitStack() as ps2:
                    psB = ps2.enter_context(
                        tc.tile_pool(name=f"psB{hf}", bufs=1, space="PSUM")
                    )
                    ps_rT = psB.tile([128, HALF], F32, tag="ps_rT")
                    for mc in range(NMC):
                        for ck in range(HALF // 512):
                            nc.tensor.matmul(
                                ps_rT[:, ck * 512 : (ck + 1) * 512],
                                Wr_sb[:, mc * HF : (mc + 1) * HF],
                                xts[mc][:, ck * 512 : (ck + 1) * 512],
                                start=(mc == 0),
                                stop=(mc == NMC - 1),
                            )
                    nc.scalar.copy(rT_sb[:, o : o + HALF], ps_rT[:])

                    ps_cj = psB.tile([128, NJT], F32, tag="ps_cj")
                    for k in range(NJT // 2):
                        nc.tensor.matmul(
                            ps_cj[:, 2 * k : 2 * k + 2],
                            hT_sb[:, k * 128 : (k + 1) * 128],
                            wj2_sb[:],
                            start=(k == 0),
                            stop=(k == NJT // 2 - 1),
                        )
                    nc.vector.tensor_copy(cjT[:, hf * NJT : (hf + 1) * NJT], ps_cj[:])

                    ps_h = psB.tile([128, HALF], BF16, tag="ps_h")
                    for k in range(HALF // 128):
                        nc.tensor.transpose(
                            ps_h[:, k * 128 : (k + 1) * 128],
                            hT_sb[:, k * 128 : (k + 1) * 128],
                            eye_sb[:],
                        )
                    # bf16 PSUM source keeps this copy in the DVE 2x path,
                    # running parallel to the rT evacuation on ACT.
                    nc.vector.tensor_copy(h_sb[:, o : o + HALF], ps_h[:])

        # ---------- Phase 2: attention ----------
        for half in range(2):
            i0 = half * HALF
            with ExitStack() as pmm_ctx:
                pmm = pmm_ctx.enter_context(
                    tc.tile_pool(name=f"pmm{half}", bufs=1, space="PSUM")
                )
                yT_ps = pmm.tile([128, HALF], F32, tag="yT")
                rs_ps = pmm.tile([1, HALF], F32, tag="rs")

                for jt in range(NJT):
                    m_t = mpool.tile([128, HALF], BF16, tag="m")
                    # Alternate DMA queues so transfers and end-of-program
                    # queue drains parallelize.
                    dma_eng = nc.gpsimd if jt % 2 == 0 else nc.sync
                    dma_eng.dma_start(
                        m_t[:], mask_d[jt * 128 : (jt + 1) * 128, i0 : i0 + HALF]
                    )
                    w_t = wpool.tile([128, HALF], BF16, tag="w")
                    nc.vector.tensor_tensor(
                        w_t[:], m_t[:], ciB[:, i0 : i0 + HALF], op=OP.add
                    )
                    z_t = zpool.tile([128, HALF], BF16, tag="z")
                    nc.scalar.activation(
                        z_t[:],
                        w_t[:],
                        AF.Prelu,
                        bias=cjT[:, 2 * jt : 2 * jt + 1],
                        alpha=SLOPE,
                    )
                    e_t = epool.tile([128, HALF], I16, tag="e")
                    nc.vector.tensor_scalar(
                        e_t[:], z_t[:], A_EXP, B_EXP, op0=OP.mult, op1=OP.add
                    )
                    e_bf = e_t[:].bitcast(BF16)
                    hr = h_sb[:, jt * 128 : (jt + 1) * 128]
                    # rs first so the final rowsum (tail-critical for the
                    # reciprocal) completes before the last yT matmuls.
                    for c in range(HALF // 512):
                        nc.tensor.matmul(
                            rs_ps[0:1, c * 512 : (c + 1) * 512],
                            ones_sb[:],
                            e_bf[:, c * 512 : (c + 1) * 512],
                            start=(jt == 0),
                            stop=(jt == NJT - 1),
                        )
                    for c in range(HALF // 512):
                        nc.tensor.matmul(
                            yT_ps[:, c * 512 : (c + 1) * 512],
                            hr,
                            e_bf[:, c * 512 : (c + 1) * 512],
                            start=(jt == 0),
                            stop=(jt == NJT - 1),
                        )

                # Finale: approx-recip the rowsums, broadcast, normalize, DMA.
                yT_sb = fin.tile([128, HALF], BF16, tag="yT_sb")
                nc.scalar.copy(yT_sb[:], yT_ps[:])
                recip_row = fin.tile([1, HALF], F32, tag="recip_row")
                nc.vector.reciprocal_approx_fast(recip_row[:], rs_ps[0:1, :])
                recipB = fin.tile([128, HALF], F32, tag="recipB")
                for c in range(4):
                    sl = slice(c * 512, (c + 1) * 512)
                    nc.gpsimd.partition_broadcast(
                        recipB[:, sl], recip_row[0:1, sl]
                    )
                ytn = fin.tile([128, HALF], BF16, tag="ytn")
                for c in range(4):
                    sl = slice(c * 512, (c + 1) * 512)
                    nc.vector.tensor_tensor(
                        ytn[:, sl], yT_sb[:, sl], recipB[:, sl], op=OP.mult
                    )
                    nc.vector.tensor_tensor(
                        ytn[:, sl],
                        ytn[:, sl],
                        rT_sb[:, i0 + c * 512 : i0 + (c + 1) * 512],
                        op=OP.add,
                    )
                    nc.scalar.dma_start(
                        y_d[:, i0 + c * 512 : i0 + (c + 1) * 512], ytn[:, sl]
                    )

    nc.compile()
    return nc


def _get_program():
    global _prog
    if _prog is None:
        _prog = build_program()
    return _prog


def _prepare_in_maps(x, graph, W, w_i, w_j, W_r):
    bf = ml_dtypes.bfloat16
    xT = np.ascontiguousarray(x.T).astype(bf)
    mask = np.where(graph > 0, np.float32(0.0), np.float32(MASK_NEG)).astype(bf)
    eye = np.eye(128, dtype=np.float32).astype(bf)
    ones = np.ones((128, 1), dtype=np.float32).astype(bf)
    in_maps = []
    for c in range(HEADS):
        wj2 = np.zeros((HF, 2), dtype=np.float32)
        wj2[:, 0] = np.asarray(w_j[c], dtype=np.float32).reshape(HF)
        in_maps.append(
            {
                "xT": xT,
                "mask": mask,
                "W": np.ascontiguousarray(W[c]).astype(bf),
                "Wr": np.ascontiguousarray(W_r[:, c * HF : (c + 1) * HF]).astype(bf),
                "Wwi": (
                    np.asarray(W[c], dtype=np.float32)
                    @ np.asarray(w_i[c], dtype=np.float32)
                ).astype(bf),
                "wj2": wj2.astype(bf),
                "ones": ones,
                "eye": eye,
            }
        )
    return in_maps


def run(inputs, trace=False, **kwargs):
    """Run the SPMD kernel; returns (y_full, BassKernelResults)."""
    x = np.asarray(inputs["x"], dtype=np.float32)
    graph = np.asarray(inputs["graph"])
    W = np.asarray(inputs["W"], dtype=np.float32)
    w_i = np.asarray(inputs["w_i"], dtype=np.float32)
    w_j = np.asarray(inputs["w_j"], dtype=np.float32)
    W_r = np.asarray(inputs["W_r"], dtype=np.float32)
    bias = np.asarray(inputs["bias"], dtype=np.float32)

    nc = _get_program()
    in_maps = _prepare_in_maps(x, graph, W, w_i, w_j, W_r)
    br = run_bass_kernel_spmd(
        nc, in_maps, core_ids=list(range(HEADS)), trace=trace, **kwargs
    )
    y = np.concatenate(
        [br.results[c]["y"].astype(np.float32).T for c in range(HEADS)], axis=1
    )
    y = y + bias[None, :]
    return y.astype(np.float32), br


def kernel(**inputs):
    y, _ = run(inputs)
    return y


# revision 38
# speedup vs baseline: 1.7837x; 1.0045x over previous
"""GAT forward on 8 Trainium2 NeuronCores — one attention head per core.

Math (per head, all [4096] nodes):
    h    = x @ W                    [N, 128]
    ci   = h @ w_i  (per-node)      [N]
    cj   = h @ w_j  (per-node)      [N]
    z    = leaky_relu(ci[i] + cj[j] + m[j,i])   (m = 0 / -60 additive mask)
    e    = exp(z)  (Schraudolph bit-trick on DVE, bf16)
    yT[f,i] = sum_j h[j,f] * e[j,i]            (PE matmul, e moving)
    rs[i]   = sum_j e[j,i]                     (PE matmul vs ones4 stationary)
    out[f,i] = yT[f,i]/rs[i] + residT[f,i];  host transposes to [i,f].

Engine assignment per attention tile [128j x 2048i] (64 tiles):
  - DVE:  w = mask + ciB          (tensor_tensor bf16, 2x mode)
          e = int16(z*A + B)      (tensor_scalar bf16->int16, Schraudolph exp;
                                   bitcast to bf16 feeds the PE directly)
  - ACT:  z = Prelu(w + cj_bias)  (one op does the cj add AND the leaky relu)
  - PE:   4x yT matmuls + 4x rowsum matmuls (all bf16, 512-col chunks)
  - Pool: mask DMA issue + ci/recip partition broadcasts
All matmuls everywhere are bf16 (1 cyc/row); phase 1 computes hT and residT
with the same streamed xT tiles. rowsum lands in one PSUM bank via a [4,512]
layout (block-diagonal ones stationary). Host does transpose/concat/bias.
"""
import sys

sys.path.insert(0, "/opt/trn_rl_repo")
from contextlib import ExitStack

import numpy as np
import ml_dtypes

import concourse.bass as bass
import concourse.tile as tile
from concourse import bacc, mybir
from concourse.bass_utils import run_bass_kernel_spmd

dt = mybir.dt
F32, BF16, I16 = dt.float32, dt.bfloat16, dt.int16
AF = mybir.ActivationFunctionType
OP = mybir.AluOpType

N = 4096
IN_F = 512
HF = 128
HEADS = 8
SLOPE = 0.2
MASK_NEG = -60.0
HALF = 2048
NJT = N // 128  # 32 j-tiles
NMC = IN_F // 128  # 4 contraction chunks
A_EXP = 184.6650292  # 128 * log2(e)
B_EXP = 16248.58  # 127*128 - schraudolph correction (round-to-nearest)

_prog = None


def build_program():
    nc = bacc.Bacc("TRN2", target_bir_lowering=False, debug=False)
    xT_d = nc.dram_tensor("xT", [IN_F, N], BF16, kind="ExternalInput").ap()
    mask_d = nc.dram_tensor("mask", [N, N], BF16, kind="ExternalInput").ap()
    W_d = nc.dram_tensor("W", [IN_F, HF], BF16, kind="ExternalInput").ap()
    Wr_d = nc.dram_tensor("Wr", [IN_F, HF], BF16, kind="ExternalInput").ap()
    Wwi_d = nc.dram_tensor("Wwi", [IN_F, 1], BF16, kind="ExternalInput").ap()
    wi_d = nc.dram_tensor("wi", [HF, 1], BF16, kind="ExternalInput").ap()
    wj2_d = nc.dram_tensor("wj2", [HF, 2], BF16, kind="ExternalInput").ap()
    ones_d = nc.dram_tensor("ones", [128, 1], BF16, kind="ExternalInput").ap()
    eye_d = nc.dram_tensor("eye", [128, 128], BF16, kind="ExternalInput").ap()
    y_d = nc.dram_tensor("y", [HF, N], BF16, kind="ExternalOutput").ap()

    with tile.TileContext(nc) as tc, ExitStack() as ctx:
        persist = ctx.enter_context(tc.tile_pool(name="persist", bufs=1))
        h_sb = persist.tile([128, N], BF16, tag="h")  # h[j,f] per j-tile
        rT_sb = persist.tile([128, N], BF16, tag="rT")  # residT[f,i]
        ciB = persist.tile([128, N], BF16, tag="ciB")  # ci bcast along partitions
        cjT = persist.tile([128, 2 * NJT], F32, tag="cjT")  # cj at even cols
        ones_sb = persist.tile([128, 1], BF16, tag="ones")
        eye_sb = persist.tile([128, 128], BF16, tag="eye")
        nc.gpsimd.dma_start(ones_sb[:], ones_d)
        nc.gpsimd.dma_start(eye_sb[:], eye_d)

        # Phase-2 pools opened first so their SBUF is disjoint from phase-1
        # scoped buffers.
        mpool = ctx.enter_context(tc.tile_pool(name="mpool", bufs=8))
        wpool = ctx.enter_context(tc.tile_pool(name="wpool", bufs=4))
        zpool = ctx.enter_context(tc.tile_pool(name="zpool", bufs=4))
        epool = ctx.enter_context(tc.tile_pool(name="epool", bufs=4))
        fin = ctx.enter_context(tc.tile_pool(name="fin", bufs=2))

        # ---------- Phase 1: hT/residT over streamed xT; ci/cj/h ----------
        with ExitStack() as p1:
            ph1 = p1.enter_context(tc.tile_pool(name="ph1", bufs=1))
            xpool = p1.enter_context(tc.tile_pool(name="xpool", bufs=5))
            hTp = p1.enter_context(tc.tile_pool(name="hTp", bufs=2))

            # Sync queue: x tiles (PE-critical) then the mask stream; all
            # small constants ride the gpsimd queue.
            W_sb = ph1.tile([128, NMC * HF], BF16, tag="W")
            Wr_sb = ph1.tile([128, NMC * HF], BF16, tag="Wr")
            Wwi_sb = ph1.tile([128, NMC], BF16, tag="Wwi")
            xt0 = xpool.tile([128, HALF], BF16, tag="xt")
            nc.sync.dma_start(xt0[:], xT_d[0:128, 0:HALF])
            for mc in range(NMC):
                nc.gpsimd.dma_start(
                    W_sb[:, mc * HF : (mc + 1) * HF], W_d[mc * 128 : (mc + 1) * 128, :]
                )
                nc.gpsimd.dma_start(
                    Wwi_sb[:, mc : mc + 1], Wwi_d[mc * 128 : (mc + 1) * 128, :]
                )
            for mc in range(NMC):
                nc.gpsimd.dma_start(
                    Wr_sb[:, mc * HF : (mc + 1) * HF],
                    Wr_d[mc * 128 : (mc + 1) * 128, :],
                )
            wi_sb = ph1.tile([128, 1], BF16, tag="wi")
            nc.gpsimd.dma_start(wi_sb[:], wi_d)
            wj2_sb = ph1.tile([128, 2], BF16, tag="wj2")
            nc.gpsimd.dma_start(wj2_sb[:], wj2_d)

            for hf in range(2):
                o = hf * HALF
                hT_sb = hTp.tile([128, HALF], BF16, tag="hT")
                xts = []
                # Loop 1: hT. For half 0 only, ci rides in-stream (via the
                # host-precomputed W@w_i) so ciB is ready early and phase-2
                # elementwise starts while phase 1 still owns the PE. Half
                # 1's ci deadline is loose, so it skips the extra 4 PSUM
                # banks here (faster pool handover from the previous loop).
                with ExitStack() as ps1:
                    psA = ps1.enter_context(
                        tc.tile_pool(name=f"psA{hf}", bufs=1, space="PSUM")
                    )
                    ps_hT = psA.tile([128, HALF], F32, tag="ps_hT")
                    ps_ci = (
                        psA.tile([1, HALF], F32, tag="ps_ci", name="ps_ci")
                        if hf == 0
                        else None
                    )
                    for mc in range(NMC):
                        if hf == 0 and mc == 0:
                            xt = xt0
                        else:
                            xt = xpool.tile([128, HALF], BF16, tag="xt")
                            nc.sync.dma_start(
                                xt[:], xT_d[mc * 128 : (mc + 1) * 128, o : o + HALF]
                            )
                        xts.append(xt)
                        for ck in range(HALF // 512):
                            nc.tensor.matmul(
                                ps_hT[:, ck * 512 : (ck + 1) * 512],
                                W_sb[:, mc * HF : (mc + 1) * HF],
                                xt[:, ck * 512 : (ck + 1) * 512],
                                start=(mc == 0),
                                stop=(mc == NMC - 1),
                            )
                        if hf == 0:
                            for ck in range(HALF // 512):
                                nc.tensor.matmul(
                                    ps_ci[0:1, ck * 512 : (ck + 1) * 512],
                                    Wwi_sb[:, mc : mc + 1],
                                    xt[:, ck * 512 : (ck + 1) * 512],
                                    start=(mc == 0),
                                    stop=(mc == NMC - 1),
                                )
                    # Evacuate on different engines so the PSUM frees fast.
                    nc.scalar.copy(hT_sb[:], ps_hT[:])
                    if hf == 0:
                        ci_row = ph1.tile([1, HALF], BF16, tag="ci_row0")
                        nc.vector.tensor_copy(ci_row[:], ps_ci[:])
                        for c in range(4):
                            nc.gpsimd.partition_broadcast(
                                ciB[:, o + c * 512 : o + (c + 1) * 512],
                                ci_row[0:1, c * 512 : (c + 1) * 512],
                            )

                # Loop 2: residT (reusing the resident x tiles), cj, h, and
                # (half 1) ci. ci comes last so its PSUM allocation lands in
                # the banks freed by the rT evacuation without stalling PE.
                with ExitStack() as ps2:
                    psB = ps2.enter_context(
                        tc.tile_pool(name=f"psB{hf}", bufs=1, space="PSUM")
                    )
                    ps_rT = psB.tile([128, HALF], F32, tag="ps_rT")
                    for mc in range(NMC):
                        for ck in range(HALF // 512):
                            nc.tensor.matmul(
                                ps_rT[:, ck * 512 : (ck + 1) * 512],
                                Wr_sb[:, mc * HF : (mc + 1) * HF],
                                xts[mc][:, ck * 512 : (ck + 1) * 512],
                                start=(mc == 0),
                                stop=(mc == NMC - 1),
                            )
                    nc.scalar.copy(rT_sb[:, o : o + HALF], ps_rT[:])

                    ps_cj = psB.tile([128, NJT], F32, tag="ps_cj")
                    for k in range(NJT // 2):
                        nc.tensor.matmul(
                            ps_cj[:, 2 * k : 2 * k + 2],
                            hT_sb[:, k * 128 : (k + 1) * 128],
                            wj2_sb[:],
                            start=(k == 0),
                            stop=(k == NJT // 2 - 1),
                        )
                    nc.vector.tensor_copy(cjT[:, hf * NJT : (hf + 1) * NJT], ps_cj[:])

                    ps_h = psB.tile([128, HALF], BF16, tag="ps_h")
                    for k in range(HALF // 128):
                        nc.tensor.transpose(
                            ps_h[:, k * 128 : (k + 1) * 128],
                            hT_sb[:, k * 128 : (k + 1) * 128],
                            eye_sb[:],
                        )
                    # bf16 PSUM source keeps this copy in the DVE 2x path,
                    # running parallel to the rT evacuation on ACT.
                    nc.vector.tensor_copy(h_sb[:, o : o + HALF], ps_h[:])

                if hf == 1:
                    with ExitStack() as ps3:
                        psC = ps3.enter_context(
                            tc.tile_pool(name="psC", bufs=1, space="PSUM")
                        )
                        ps_ci1 = psC.tile([1, HALF], F32, tag="ps_ci1")
                        for c in range(4):
                            nc.tensor.matmul(
                                ps_ci1[0:1, c * 512 : (c + 1) * 512],
                                wi_sb[:],
                                hT_sb[:, c * 512 : (c + 1) * 512],
                                start=True,
                                stop=True,
                            )
                        ci_row1 = ph1.tile([1, HALF], BF16, tag="ci_row1")
                        nc.vector.tensor_copy(ci_row1[:], ps_ci1[:])
                        for c in range(4):
                            nc.gpsimd.partition_broadcast(
                                ciB[:, o + c * 512 : o + (c + 1) * 512],
                                ci_row1[0:1, c * 512 : (c + 1) * 512],
                            )

        # ---------- Phase 2: attention ----------
        for half in range(2):
            i0 = half * HALF
            with ExitStack() as pmm_ctx:
                pmm = pmm_ctx.enter_context(
                    tc.tile_pool(name=f"pmm{half}", bufs=1, space="PSUM")
                )
                # yT as four chunk tiles so boundary evacuations pipeline
                # bank-by-bank instead of waiting on one big tile.
                yT_ps = [
                    pmm.tile([128, 512], F32, tag=f"yT{c}", name=f"yT_ps{c}")
                    for c in range(4)
                ]
                rs_ps = pmm.tile([1, HALF], F32, tag="rs")

                for jt in range(NJT):
                    m_t = mpool.tile([128, HALF], BF16, tag="m")
                    nc.sync.dma_start(
                        m_t[:], mask_d[jt * 128 : (jt + 1) * 128, i0 : i0 + HALF]
                    )
                    w_t = wpool.tile([128, HALF], BF16, tag="w")
                    nc.vector.tensor_tensor(
                        w_t[:], m_t[:], ciB[:, i0 : i0 + HALF], op=OP.add
                    )
                    z_t = zpool.tile([128, HALF], BF16, tag="z")
                    nc.scalar.activation(
                        z_t[:],
                        w_t[:],
                        AF.Prelu,
                        bias=cjT[:, 2 * jt : 2 * jt + 1],
                        alpha=SLOPE,
                    )
                    e_t = epool.tile([128, HALF], I16, tag="e")
                    nc.vector.tensor_scalar(
                        e_t[:], z_t[:], A_EXP, B_EXP, op0=OP.mult, op1=OP.add
                    )
                    e_bf = e_t[:].bitcast(BF16)
                    hr = h_sb[:, jt * 128 : (jt + 1) * 128]
                    # rs first so the final rowsum (tail-critical for the
                    # reciprocal) completes before the last yT matmuls.
                    for c in range(HALF // 512):
                        nc.tensor.matmul(
                            rs_ps[0:1, c * 512 : (c + 1) * 512],
                            ones_sb[:],
                            e_bf[:, c * 512 : (c + 1) * 512],
                            start=(jt == 0),
                            stop=(jt == NJT - 1),
                        )
                    for c in range(HALF // 512):
                        nc.tensor.matmul(
                            yT_ps[c][:],
                            hr,
                            e_bf[:, c * 512 : (c + 1) * 512],
                            start=(jt == 0),
                            stop=(jt == NJT - 1),
                        )

                # Finale: approx-recip the rowsums, broadcast, normalize, DMA.
                yT_sb = fin.tile([128, HALF], BF16, tag="yT_sb")
                for c in range(4):
                    nc.scalar.copy(yT_sb[:, c * 512 : (c + 1) * 512], yT_ps[c][:])
                recip_row = fin.tile([1, HALF], F32, tag="recip_row")
                nc.vector.reciprocal_approx_fast(recip_row[:], rs_ps[0:1, :])
                recipB = fin.tile([128, HALF], F32, tag="recipB")
                for c in range(4):
                    sl = slice(c * 512, (c + 1) * 512)
                    nc.gpsimd.partition_broadcast(
                        recipB[:, sl], recip_row[0:1, sl]
                    )
                ytn = fin.tile([128, HALF], BF16, tag="ytn")
                for c in range(4):
                    sl = slice(c * 512, (c + 1) * 512)
                    nc.vector.tensor_tensor(
                        ytn[:, sl], yT_sb[:, sl], recipB[:, sl], op=OP.mult
                    )
                    nc.vector.tensor_tensor(
                        ytn[:, sl],
                        ytn[:, sl],
                        rT_sb[:, i0 + c * 512 : i0 + (c + 1) * 512],
                        op=OP.add,
                    )
                    nc.scalar.dma_start(
                        y_d[:, i0 + c * 512 : i0 + (c + 1) * 512], ytn[:, sl]
                    )

    nc.compile()
    return nc


def _get_program():
    global _prog
    if _prog is None:
        _prog = build_program()
    return _prog


def _prepare_in_maps(x, graph, W, w_i, w_j, W_r):
    bf = ml_dtypes.bfloat16
    xT = np.ascontiguousarray(x.T).astype(bf)
    mask = np.where(graph > 0, np.float32(0.0), np.float32(MASK_NEG)).astype(bf)
    eye = np.eye(128, dtype=np.float32).astype(bf)
    ones = np.ones((128, 1), dtype=np.float32).astype(bf)
    in_maps = []
    for c in range(HEADS):
        wj2 = np.zeros((HF, 2), dtype=np.float32)
        wj2[:, 0] = np.asarray(w_j[c], dtype=np.float32).reshape(HF)
        in_maps.append(
            {
                "xT": xT,
                "mask": mask,
                "W": np.ascontiguousarray(W[c]).astype(bf),
                "Wr": np.ascontiguousarray(W_r[:, c * HF : (c + 1) * HF]).astype(bf),
                "Wwi": (
                    np.asarray(W[c], dtype=np.float32)
                    @ np.asarray(w_i[c], dtype=np.float32)
                ).astype(bf),
                "wi": np.asarray(w_i[c], dtype=np.float32).astype(bf),
                "wj2": wj2.astype(bf),
                "ones": ones,
                "eye": eye,
            }
        )
    return in_maps


def run(inputs, trace=False, **kwargs):
    """Run the SPMD kernel; returns (y_full, BassKernelResults)."""
    x = np.asarray(inputs["x"], dtype=np.float32)
    graph = np.asarray(inputs["graph"])
    W = np.asarray(inputs["W"], dtype=np.float32)
    w_i = np.asarray(inputs["w_i"], dtype=np.float32)
    w_j = np.asarray(inputs["w_j"], dtype=np.float32)
    W_r = np.asarray(inputs["W_r"], dtype=np.float32)
    bias = np.asarray(inputs["bias"], dtype=np.float32)

    nc = _get_program()
    in_maps = _prepare_in_maps(x, graph, W, w_i, w_j, W_r)
    br = run_bass_kernel_spmd(
        nc, in_maps, core_ids=list(range(HEADS)), trace=trace, **kwargs
    )
    y = np.concatenate(
        [br.results[c]["y"].astype(np.float32).T for c in range(HEADS)], axis=1
    )
    y = y + bias[None, :]
    return y.astype(np.float32), br


def kernel(**inputs):
    y, _ = run(inputs)
    return y
